# revision 74
# baseline (speedup 1.0000x reference)
# Trainium2 Bass kernel for AttentionPooling (segment softmax-pool).
#
# Math: reference's per-slot max subtraction cancels in the softmax, so
#   w[t,k] = exp(s_t) / D_k,  D_k = sum_{t in slot_k} exp(s_t)
#   out[k,:] = sum_{t in slot_k} exp(s_t) * proj[t,:] / D_k
# (b2 shifts every score equally so it cancels too and is dropped.)
# With A[t,k] = in_slot(t,k) * exp(s_t), both numerator and D come from one
# accumulated PE matmul per 128-row chunk:  [num | D] += A^T @ [proj | 1].
#
# The score MLP needs proj with H on partitions. Two sources are mixed:
#  - an fp8e4m3 h-major copy shipped from HBM, consumed by a DoubleRow W1
#    matmul (256-row contraction at 0.5 cyc/col);
#  - for mid-stream jobs, the first TN chunks are instead TRANSPOSED on the
#    PE from the already-resident t-major seg tile (identity matmul), copied
#    PSUM->SBUF, and run through a bf16 W1 — trading idle PE/DVE/Act time
#    for ~3.3us less HBM traffic.
# The transpose path is software-pipelined across loop iterations (pt fetch
# at jx-3, g fetch at jx-2, transpose+fp8 matmul at jx-1, copy at jx, bf16
# MLP at jx+1, masks at jx+2, seg matmuls at jx+3) so every engine's ops at
# each iteration depend only on results >= 1 DMA beat old — the in-order
# sequencers never head-of-line block.
#
# Scores only feed exp(); fp8/bf16 mixing keeps rel err ~9e-3 vs the 2e-2
# gate. Slot masks are e*(start<=t) - e*(end<=t): one 2K-wide int16 compare
# (DVE 2x mode) + a bf16 subtract, split 4:1 DVE:GpSimd.
#
# Sharding: data-parallel over B; core i handles batches 2i, 2i+1.

import numpy as np
import ml_dtypes

import concourse.bacc as bacc
import concourse.tile as tile
import concourse.mybir as mybir
import concourse.bass as bass
from concourse.bass_utils import run_bass_kernel_spmd

B, T, H, K = 16, 8192, 256, 128
HQ = 64
NCORES = 8
BPC = B // NCORES          # batches per core
CH = 128                   # rows per chunk
NCH = T // CH              # 64 chunks per batch
GRP = 8                    # chunks per DMA job
SUB = 4                    # chunks per W1-matmul/tanh subgroup

F32 = mybir.dt.float32
BF16 = mybir.dt.bfloat16
F16 = mybir.dt.float16
I16 = mybir.dt.int16
FP8 = mybir.dt.float8e4

# chunk c's mask-gen runs on Pool (gpsimd) when c % POOL_MOD < POOL_TAKE
POOL_MOD = 5
POOL_TAKE = 1
# mid-stream jobs: first TN chunks' scores via on-chip PE transpose
TN = 4
TR_ENABLE = False
TR_ONLY = {5}


def make_jobs():
    """Job list in PE/accumulation order: (b, c0, n, preloaded)."""
    jobs = []
    last_b = BPC - 1
    c_pre = NCH - GRP            # 56: preloaded group (first in PE order)
    c_str = NCH - 2 * GRP        # 48: tapered streamed group (last)
    for b in range(BPC):
        NG = NCH // GRP
        if b == last_b:
            jobs.append((b, c_pre, GRP, True))
            for G in range(NG - 2):
                jobs.append((b, G * GRP, GRP, False))
            jobs += [
                (b, c_str, 4, False),
                (b, c_str + 4, 2, False),
                (b, c_str + 6, 1, False),
                (b, c_str + 7, 1, False),
            ]
        else:
            for G in range(NG):
                jobs.append((b, G * GRP, GRP, False))
    return jobs


def build_program():
    nc = bacc.Bacc(None, target_bir_lowering=False, debug=False)

    NG = NCH // GRP
    proj = nc.dram_tensor("proj", [BPC, NG, CH, GRP, H + 1], BF16, kind="ExternalInput")
    projq = nc.dram_tensor("projq", [BPC, CH, 2, T], FP8, kind="ExternalInput")
    bounds = nc.dram_tensor("bounds", [2, BPC, K], I16, kind="ExternalInput")
    wpack = nc.dram_tensor("wpack", [CH, 2, HQ], FP8, kind="ExternalInput")
    if TR_ENABLE:
        wpb = nc.dram_tensor("wpb", [CH, 2, HQ], BF16, kind="ExternalInput")
        identm = nc.dram_tensor("identm", [CH, CH], BF16, kind="ExternalInput")
    w2t = nc.dram_tensor("w2t", [HQ], BF16, kind="ExternalInput")
    b1 = nc.dram_tensor("b1", [HQ], F32, kind="ExternalInput")
    # raw [num | D] per slot, f16; host divides num/D
    out = nc.dram_tensor("out", [BPC, K, H + 1], F16, kind="ExternalOutput")

    with tile.TileContext(nc) as tc:
        with (
            tc.tile_pool(name="const", bufs=1) as const,
            tc.tile_pool(name="projg", bufs=10) as projp,
            tc.tile_pool(name="projtg", bufs=10) as ptp,
            tc.tile_pool(name="htanh", bufs=6) as htp,
            tc.tile_pool(name="amask", bufs=20) as apool,
            tc.tile_pool(name="eall", bufs=2) as epool,
            tc.tile_pool(name="outs", bufs=2) as outp,
            tc.tile_pool(name="trsbp", bufs=3) as trp,
            tc.tile_pool(name="psH", bufs=3, space="PSUM") as psH,
            tc.tile_pool(name="psS", bufs=1, space="PSUM") as psS,
            tc.tile_pool(name="psT", bufs=2, space="PSUM") as psT,
            tc.tile_pool(name="psSeg", bufs=2, space="PSUM") as psSeg,
        ):
            COPY_ENGS = [nc.vector, nc.scalar]
            jobs = make_jobs()
            last_issued = {}
            first_issued = {}
            last_jx = {}
            for jx, (b_, c0_, n_, _pre) in enumerate(jobs):
                last_issued[b_] = c0_ + n_ - 1
                last_jx[b_] = jx
                if b_ not in first_issued:
                    first_issued[b_] = c0_
            e_alls = [
                epool.tile([CH, NCH], F32, tag="eall", name=f"e_all{b}")
                for b in range(BPC)
            ]
            segs = [
                psSeg.tile([K, H + 1], F32, tag="seg", name=f"seg{b}")
                for b in range(BPC)
            ]
            # one shared score-PSUM tile; jobs rotate over 3 column slots
            s_ps_all = psS.tile([CH, 4 * GRP], F32, tag="spsall", name="s_ps_all")
            slot_ctr = [0]

            def dma_pt(b, c0, n):
                pt_tile = ptp.tile([CH, 2, GRP * CH], FP8, tag="pt", name="pt_tile")
                nc.sync.dma_start(
                    out=pt_tile[:, :, 0 : n * CH],
                    in_=bass.AP(
                        projq,
                        b * CH * 2 * T + c0 * CH,
                        [[2 * T, CH], [T, 2], [1, n * CH]],
                    ),
                )
                return pt_tile

            # kick off the first score slab before the constant loads
            pt_tiles = {0: dma_pt(*jobs[0][:3])}

            # ---- constants ----
            tcol = const.tile([CH, NCH], F32)
            nc.gpsimd.iota(
                tcol[:],
                pattern=[[CH, NCH]],
                base=0,
                channel_multiplier=1,
                allow_small_or_imprecise_dtypes=True,
            )
            # wp is fetched on the SP queue after the lead pt fetches: the
            # first W1 matmul has ~3us of slack, and keeping wp off the head
            # of the HWDGE queue closes a ~400ns stream gap
            wp = const.tile([CH, 2, HQ], FP8)
            w2_sb = const.tile([HQ, 1], BF16)
            nc.gpsimd.dma_start(out=w2_sb[:], in_=bass.AP(w2t, 0, [[1, HQ], [1, 1]]))
            b1_sb = const.tile([HQ, 1], F32)
            nc.gpsimd.dma_start(out=b1_sb[:], in_=bass.AP(b1, 0, [[1, HQ], [1, 1]]))
            bnd = const.tile([CH, 2, BPC, K], I16)
            nc.gpsimd.dma_start(
                out=bnd[:],
                in_=bass.AP(bounds, 0, [[0, CH], [BPC * K, 2], [K, BPC], [1, K]]),
            )
            if TR_ENABLE:
                wpb_sb = const.tile([CH, 2, HQ], BF16)
                nc.gpsimd.dma_start(
                    out=wpb_sb[:],
                    in_=bass.AP(wpb, 0, [[2 * HQ, CH], [HQ, 2], [1, HQ]]),
                )
                ident = const.tile([CH, CH], BF16)
                nc.gpsimd.dma_start(
                    out=ident[:], in_=bass.AP(identm, 0, [[CH, CH], [1, CH]])
                )

            def scores_part(pt_tile, pt_off, n, sbase):
                # fp8 DoubleRow W1 + tanh + per-chunk W2 into s_ps_all cols
                # [sbase, sbase+n)
                for s0 in range(0, n, SUB):
                    ns = min(SUB, n - s0)
                    po = pt_off + s0
                    hps = psH.tile([HQ, SUB * CH], F32, tag="hps", name="hps")
                    nc.tensor.matmul(
                        hps[:, 0 : ns * CH],
                        wp[:],
                        pt_tile[:, :, po * CH : (po + ns) * CH],
                        start=True,
                        stop=True,
                        perf_mode=mybir.MatmulPerfMode.DoubleRow,
                    )
                    hts = htp.tile([HQ, SUB * CH], BF16, tag="hts", name="hts")
                    nc.scalar.activation(
                        out=hts[:, 0 : ns * CH],
                        in_=hps[:, 0 : ns * CH],
                        func=mybir.ActivationFunctionType.Tanh,
                        bias=b1_sb[:],
                        scale=1.0,
                    )
                    for j in range(ns):
                        nc.tensor.matmul(
                            s_ps_all[:, sbase + s0 + j : sbase + s0 + j + 1],
                            hts[:, j * CH : (j + 1) * CH],
                            w2_sb[:],
                            start=True,
                            stop=True,
                        )

            def scores(b, c0, n, pt_tile, pt_off=0):
                slot = slot_ctr[0] % 4
                slot_ctr[0] += 1
                sbase = slot * GRP
                scores_part(pt_tile, pt_off, n, sbase)
                nc.scalar.activation(
                    out=e_alls[b][:, c0 : c0 + n],
                    in_=s_ps_all[:, sbase : sbase + n],
                    func=mybir.ActivationFunctionType.Exp,
                )

            # --- pipelined transpose-score stages for tr jobs ---
            tr_state = {}

            def tr_stage1(jx):
                # fp8 part (pt fetched 2 beats ago) + PE transposes of the
                # g tile (fetched 1 beat ago) into a bf16 PSUM tile
                b, c0, n, _ = jobs[jx]
                slot = slot_ctr[0] % 4
                slot_ctr[0] += 1
                sbase = slot * GRP
                scores_part(pt_tiles.pop(jx), 0, GRP - TN, sbase + TN)
                g_tile = tr_state[jx]["g"]
                psT_t = psT.tile(
                    [CH, TN * 2 * CH], BF16, tag="pst", name="psT_t"
                )
                for g in range(TN):
                    for h in range(2):
                        nc.tensor.matmul(
                            psT_t[:, (2 * g + h) * CH : (2 * g + h + 1) * CH],
                            g_tile[:, g, h * CH : (h + 1) * CH],
                            ident[:],
                            start=True,
                            stop=True,
                            is_transpose=True,
                        )
                tr_state[jx]["psT"] = psT_t
                tr_state[jx]["sbase"] = sbase

            def tr_stage2(jx):
                # PSUM -> SBUF copy of the transposed halves
                trsb = trp.tile([CH, TN * 2 * CH], BF16, tag="tr", name="trsb")
                eng = COPY_ENGS[jx % len(COPY_ENGS)]
                if eng is nc.scalar:
                    eng.copy(out=trsb[:], in_=tr_state[jx].pop("psT")[:])
                else:
                    eng.tensor_copy(trsb[:], tr_state[jx].pop("psT")[:])
                tr_state[jx]["trsb"] = trsb

            def tr_stage3(jx):
                # bf16 W1 (2 accumulating halves per chunk) + tanh + W2 + exp
                b, c0, n, _ = jobs[jx]
                sbase = tr_state[jx]["sbase"]
                trsb = tr_state[jx].pop("trsb")
                hps = psH.tile([HQ, TN * CH], F32, tag="hps", name="hps_tr")
                for g in range(TN):
                    for h in range(2):
                        nc.tensor.matmul(
                            hps[:, g * CH : (g + 1) * CH],
                            wpb_sb[:, h, :],
                            trsb[:, (2 * g + h) * CH : (2 * g + h + 1) * CH],
                            start=(h == 0),
                            stop=(h == 1),
                        )
                hts = htp.tile([HQ, TN * CH], BF16, tag="hts", name="hts_tr")
                nc.scalar.activation(
                    out=hts[:],
                    in_=hps[:],
                    func=mybir.ActivationFunctionType.Tanh,
                    bias=b1_sb[:],
                    scale=1.0,
                )
                for g in range(TN):
                    nc.tensor.matmul(
                        s_ps_all[:, sbase + g : sbase + g + 1],
                        hts[:, g * CH : (g + 1) * CH],
                        w2_sb[:],
                        start=True,
                        stop=True,
                    )
                nc.scalar.activation(
                    out=e_alls[b][:, c0 : c0 + GRP],
                    in_=s_ps_all[:, sbase : sbase + GRP],
                    func=mybir.ActivationFunctionType.Exp,
                )

            def agen(b, c0, n, hold=False):
                e_all = e_alls[b]
                a2s = []
                for g in range(n):
                    c = c0 + g
                    eng = nc.gpsimd if (c % POOL_MOD < POOL_TAKE) else nc.vector
                    cmp = apool.tile([CH, 2, K], BF16, tag="a1", bufs=10, name="cmp")
                    a2 = apool.tile(
                        [CH, K], BF16, tag="a2h" if hold else "a2",
                        bufs=16 if hold else 32, name="a2",
                    )
                    eng.tensor_scalar(
                        out=cmp[:],
                        in0=bnd[:, :, b, :],
                        scalar1=tcol[:, c : c + 1],
                        scalar2=e_all[:, c : c + 1],
                        op0=mybir.AluOpType.is_le,
                        op1=mybir.AluOpType.mult,
                    )
                    eng.tensor_tensor(
                        out=a2[:],
                        in0=cmp[:, 0, :],
                        in1=cmp[:, 1, :],
                        op=mybir.AluOpType.subtract,
                    )
                    a2s.append(a2)
                return a2s

            def dma_g(b, c0, n, tag="g", bufs=None):
                HP = H + 1
                g_tile = projp.tile(
                    [CH, GRP, HP], BF16, tag=tag, name="g_tile",
                    **({"bufs": bufs} if bufs else {}),
                )
                G, g0 = c0 // GRP, c0 % GRP
                nc.sync.dma_start(
                    out=g_tile[:, 0:n, :],
                    in_=bass.AP(
                        proj,
                        (b * (NCH // GRP) + G) * CH * GRP * HP + g0 * HP,
                        [[GRP * HP, CH], [HP, n], [1, HP]],
                    ),
                )
                return g_tile

            def seg_mms(b, c0, n, a2s, g_tile):
                seg = segs[b]
                for g in range(n):
                    c = c0 + g
                    nc.tensor.matmul(
                        seg[:],
                        a2s[g][:],
                        g_tile[:, g, :],
                        start=(c == first_issued[b]),
                        stop=(c == last_issued[b]),
                    )

            def epilogue(b):
                seg = segs[b]
                ot = outp.tile([K, H + 1], F16, name=f"ot{b}")
                nc.scalar.copy(out=ot[:], in_=seg[:])
                return ot

            def out_dma(b, ot):
                nc.sync.dma_start(
                    out=bass.AP(out, b * K * (H + 1), [[H + 1, K], [1, H + 1]]),
                    in_=ot[:],
                )

            # ---- static schedule ----
            pre_set = [jx for jx, j in enumerate(jobs) if j[3]]
            taper_set = [
                jx for jx, j in enumerate(jobs) if not j[3] and j[2] < GRP
            ]
            lead_set = pre_set + taper_set
            last_b = BPC - 1
            c_tap = min(jobs[jx][1] for jx in taper_set)
            # tr jobs: full stream jobs except the first two and each batch's
            # last full stream job (tail safety)
            tr_set = {
                jx
                for jx, j in enumerate(jobs)
                if j[2] == GRP and not j[3] and jx not in (0, 1, 2)
            } if TR_ENABLE else set()
            if TR_ENABLE and TR_ONLY is not None:
                tr_set &= TR_ONLY
            for b_ in range(BPC):
                tr_set.discard(
                    max(
                        jx
                        for jx, j in enumerate(jobs)
                        if j[0] == b_ and j[2] == GRP and not j[3]
                    )
                )
            a2_map = {}
            g_pre = {}
            # lead: scores+masks for job0, the preloaded job, the taper jobs
            # (one fused pass) and stream jobs 1, 2
            for sx in pre_set:
                pt_tiles[sx] = dma_pt(*jobs[sx][:3])
            pt_tap = dma_pt(last_b, c_tap, GRP)
            nc.sync.dma_start(
                out=wp[:],
                in_=bass.AP(wpack, 0, [[2 * HQ, CH], [HQ, 2], [1, HQ]]),
            )
            scores(*jobs[0][:3], pt_tiles.pop(0))
            for sx in pre_set:
                scores(*jobs[sx][:3], pt_tiles.pop(sx))
                a2_map[sx] = agen(*jobs[sx][:3], hold=True)
                g_pre[sx] = dma_g(*jobs[sx][:3], tag="gpre", bufs=2)
            scores(last_b, c_tap, GRP, pt_tap)
            for sx in taper_set:
                a2_map[sx] = agen(*jobs[sx][:3], hold=True)
            for sx in (1, 2):
                pt_tiles[sx] = dma_pt(*jobs[sx][:3])
                scores(*jobs[sx][:3], pt_tiles.pop(sx))
            # fp8-scored stream jobs outside the lead
            sched_pt, sched_sc, sched_agen = {}, {}, {}
            for jx, j in enumerate(jobs):
                if (
                    jx > 2
                    and jx not in tr_set
                    and jx not in lead_set
                    and not j[3]
                ):
                    sched_pt.setdefault(max(jx - 4, 0), []).append(jx)
                    sched_sc.setdefault(max(jx - 3, 0), []).append(jx)
                    sched_agen.setdefault(max(jx - 2, 0), []).append(jx)
            for aj in (0, 1, 2):
                if aj not in a2_map:
                    a2_map[aj] = agen(*jobs[aj][:3])

            def seg_at(jx):
                # jobs after the first tr job defer their seg matmuls 3 beats
                # (keeps per-batch accumulation order monotone)
                if jx in (0, 1, 2) or jobs[jx][3]:
                    return jx
                return jx + 3

            emit_seg = {}
            for jx in range(len(jobs)):
                emit_seg.setdefault(seg_at(jx), []).append(jx)
            copy_at = {seg_at(last_jx[b_]) + 2: b_ for b_ in range(BPC)}
            dma_at = {seg_at(last_jx[b_]) + 4: b_ for b_ in range(BPC)}

            ots = {}
            g_map = {}
            NITER = len(jobs) + 4
            for i in range(NITER):
                # non-tr stream jobs fetch their g at their own iteration,
                # before any seg emission that might consume it this beat
                if i < len(jobs) and i not in tr_set and not jobs[i][3]:
                    g_map[i] = dma_g(*jobs[i][:3])
                for jx in sorted(emit_seg.pop(i, [])):
                    b, c0, n, pre = jobs[jx]
                    if pre:
                        g_tile = g_pre.pop(jx)
                    elif jx in tr_set:
                        g_tile = tr_state.pop(jx)["g"]
                    else:
                        g_tile = g_map.pop(jx)
                    seg_mms(b, c0, n, a2_map.pop(jx), g_tile)
                for sx in sched_pt.pop(i, []):
                    pt_tiles[sx] = dma_pt(*jobs[sx][:3])
                for sx in sched_sc.pop(i, []):
                    scores(*jobs[sx][:3], pt_tiles.pop(sx))
                for sx in sched_agen.pop(i, []):
                    a2_map[sx] = agen(*jobs[sx][:3])
                j = i + 3
                if j < len(jobs) and j in tr_set:
                    pt_tiles[j] = dma_pt(jobs[j][0], jobs[j][1] + TN, GRP - TN)
                j = i + 2
                if j < len(jobs) and j in tr_set:
                    tr_state[j] = {"g": dma_g(*jobs[j][:3])}
                j = i - 1
                if j >= 0 and j in tr_set:
                    tr_stage3(j)
                j = i + 1
                if j < len(jobs) and j in tr_set:
                    tr_stage1(j)
                if i in tr_set:
                    tr_stage2(i)
                j = i - 2
                if j >= 0 and j in tr_set:
                    a2_map[j] = agen(*jobs[j][:3])
                if i in copy_at:
                    ots[copy_at[i]] = epilogue(copy_at[i])
                if i in dma_at:
                    out_dma(dma_at[i], ots[dma_at[i]])
            for b_ in range(BPC):
                if b_ not in ots:
                    ots[b_] = epilogue(b_)
                    out_dma(b_, ots[b_])

    nc.compile()
    return nc


_prog_cache = None
LAST_RESULTS = None


def _get_program():
    global _prog_cache
    if _prog_cache is None:
        _prog_cache = build_program()
    return _prog_cache


def kernel(**inputs):
    proj = np.asarray(inputs["projected"], dtype=np.float32)
    bnds = np.asarray(inputs["boundaries"])
    slot = np.asarray(inputs["slot_mask"])
    W1 = np.asarray(inputs["W1"], dtype=np.float32)
    b1 = np.ascontiguousarray(np.asarray(inputs["b1"], dtype=np.float32))
    W2 = np.asarray(inputs["W2"], dtype=np.float32).reshape(HQ)

    live = slot > 0
    starts = np.where(live, bnds[..., 0], 0).astype(np.int16)     # [B, K]
    ends = np.where(live, bnds[..., 1], 0).astype(np.int16)

    # h-major fp8 for scores: [B, p, half, T]
    projq = np.ascontiguousarray(
        proj.astype(ml_dtypes.float8_e4m3)
        .transpose(0, 2, 1)
        .reshape(B, 2, CH, T)
        .transpose(0, 2, 1, 3)
    )
    # [B, T, H+1] (ones col baked in) -> [B, G, p, g, h]
    proj_bf = np.empty((B, T, H + 1), dtype=ml_dtypes.bfloat16)
    proj_bf[:, :, :H] = proj
    proj_bf[:, :, H] = 1.0
    proj_bf = np.ascontiguousarray(
        proj_bf.reshape(B, NCH // GRP, GRP, CH, H + 1).transpose(0, 1, 3, 2, 4)
    )

    # W1 packed for DoubleRow (fp8) and the transpose path (bf16)
    w1h = np.ascontiguousarray(W1.reshape(2, CH, HQ).transpose(1, 0, 2))
    wpack = w1h.astype(ml_dtypes.float8_e4m3)
    wpb = w1h.astype(ml_dtypes.bfloat16)
    identm = np.eye(CH, dtype=ml_dtypes.bfloat16)
    w2t = W2.astype(ml_dtypes.bfloat16)

    nc = _get_program()
    in_maps = []
    for i in range(NCORES):
        lo, hi = i * BPC, (i + 1) * BPC
        in_maps.append(
            {
                "proj": proj_bf[lo:hi],
                "projq": projq[lo:hi],
                "bounds": np.ascontiguousarray(
                    np.stack([starts[lo:hi], ends[lo:hi]])
                ),
                "wpack": wpack,
                **({"wpb": wpb, "identm": identm} if TR_ENABLE else {}),
                "w2t": w2t,
                "b1": b1,
            }
        )

    res = run_bass_kernel_spmd(nc, in_maps, core_ids=list(range(NCORES)))
    global LAST_RESULTS
    LAST_RESULTS = res
    raw = np.concatenate(
        [np.asarray(r["out"]) for r in res.results], axis=0
    ).astype(np.float32)                                           # [B, K, H+1]
    num, den = raw[..., :H], raw[..., H:]
    return num / np.where(den > 0, den, 1.0)


# revision 75
# speedup vs baseline: 1.0177x; 1.0177x over previous
# Trainium2 Bass kernel for AttentionPooling (segment softmax-pool).
#
# Math: reference's per-slot max subtraction cancels in the softmax, so
#   w[t,k] = exp(s_t) / D_k,  D_k = sum_{t in slot_k} exp(s_t)
#   out[k,:] = sum_{t in slot_k} exp(s_t) * proj[t,:] / D_k
# (b2 shifts every score equally so it cancels too and is dropped.)
# With A[t,k] = in_slot(t,k) * exp(s_t), both numerator and D come from one
# accumulated PE matmul per 128-row chunk:  [num | D] += A^T @ [proj | 1].
#
# The score MLP needs proj with H on partitions. Two sources are mixed:
#  - an fp8e4m3 h-major copy shipped from HBM, consumed by a DoubleRow W1
#    matmul (256-row contraction at 0.5 cyc/col);
#  - for mid-stream jobs, the first TN chunks are instead TRANSPOSED on the
#    PE from the already-resident t-major seg tile (identity matmul), copied
#    PSUM->SBUF, and run through a bf16 W1 — trading idle PE/DVE/Act time
#    for ~3.3us less HBM traffic.
# The transpose path is software-pipelined across loop iterations (pt fetch
# at jx-3, g fetch at jx-2, transpose+fp8 matmul at jx-1, copy at jx, bf16
# MLP at jx+1, masks at jx+2, seg matmuls at jx+3) so every engine's ops at
# each iteration depend only on results >= 1 DMA beat old — the in-order
# sequencers never head-of-line block.
#
# Scores only feed exp(); fp8/bf16 mixing keeps rel err ~9e-3 vs the 2e-2
# gate. Slot masks are e*(start<=t) - e*(end<=t): one 2K-wide int16 compare
# (DVE 2x mode) + a bf16 subtract, split 4:1 DVE:GpSimd.
#
# Sharding: data-parallel over B; core i handles batches 2i, 2i+1.

import numpy as np
import ml_dtypes

import concourse.bacc as bacc
import concourse.tile as tile
import concourse.mybir as mybir
import concourse.bass as bass
from concourse.bass_utils import run_bass_kernel_spmd

B, T, H, K = 16, 8192, 256, 128
HQ = 64
NCORES = 8
BPC = B // NCORES          # batches per core
CH = 128                   # rows per chunk
NCH = T // CH              # 64 chunks per batch
GRP = 8                    # chunks per DMA job
SUB = 4                    # chunks per W1-matmul/tanh subgroup

F32 = mybir.dt.float32
BF16 = mybir.dt.bfloat16
F16 = mybir.dt.float16
I16 = mybir.dt.int16
FP8 = mybir.dt.float8e4

# chunk c's mask-gen runs on Pool (gpsimd) when c % POOL_MOD < POOL_TAKE
POOL_MOD = 5
POOL_TAKE = 1
# mid-stream jobs: first TN chunks' scores via on-chip PE transpose
TN = 4
TR_ENABLE = False
TR_ONLY = {5}


def make_jobs():
    """Job list in PE/accumulation order: (b, c0, n, preloaded)."""
    jobs = []
    last_b = BPC - 1
    c_pre = NCH - GRP            # 56: preloaded group (first in PE order)
    c_str = NCH - 2 * GRP        # 48: tapered streamed group (last)
    for b in range(BPC):
        NG = NCH // GRP
        if b == last_b:
            jobs.append((b, c_pre, GRP, True))
            for G in range(NG - 2):
                jobs.append((b, G * GRP, GRP, False))
            jobs += [
                (b, c_str, 4, False),
                (b, c_str + 4, 2, False),
                (b, c_str + 6, 1, False),
                (b, c_str + 7, 1, False),
            ]
        else:
            for G in range(NG):
                jobs.append((b, G * GRP, GRP, False))
    return jobs


def build_program():
    nc = bacc.Bacc(None, target_bir_lowering=False, debug=False)

    NG = NCH // GRP
    proj = nc.dram_tensor("proj", [BPC, NG, CH, GRP, H + 1], BF16, kind="ExternalInput")
    projq = nc.dram_tensor("projq", [BPC, CH, 2, T], FP8, kind="ExternalInput")
    bounds = nc.dram_tensor("bounds", [2, BPC, K], I16, kind="ExternalInput")
    wpack = nc.dram_tensor("wpack", [CH, 2, HQ], FP8, kind="ExternalInput")
    if TR_ENABLE:
        wpb = nc.dram_tensor("wpb", [CH, 2, HQ], BF16, kind="ExternalInput")
        identm = nc.dram_tensor("identm", [CH, CH], BF16, kind="ExternalInput")
    w2t = nc.dram_tensor("w2t", [HQ], BF16, kind="ExternalInput")
    b1 = nc.dram_tensor("b1", [HQ], F32, kind="ExternalInput")
    # raw [num | D] per slot, f16; host divides num/D
    out = nc.dram_tensor("out", [BPC, K, H + 1], F16, kind="ExternalOutput")

    with tile.TileContext(nc) as tc:
        with (
            tc.tile_pool(name="const", bufs=1) as const,
            tc.tile_pool(name="projg", bufs=10) as projp,
            tc.tile_pool(name="projtg", bufs=10) as ptp,
            tc.tile_pool(name="htanh", bufs=6) as htp,
            tc.tile_pool(name="amask", bufs=20) as apool,
            tc.tile_pool(name="eall", bufs=2) as epool,
            tc.tile_pool(name="outs", bufs=2) as outp,
            tc.tile_pool(name="trsbp", bufs=3) as trp,
            tc.tile_pool(name="psH", bufs=3, space="PSUM") as psH,
            tc.tile_pool(name="psS", bufs=1, space="PSUM") as psS,
            tc.tile_pool(name="psT", bufs=2, space="PSUM") as psT,
            tc.tile_pool(name="psSeg", bufs=2, space="PSUM") as psSeg,
        ):
            COPY_ENGS = [nc.vector, nc.scalar]
            jobs = make_jobs()
            last_issued = {}
            first_issued = {}
            last_jx = {}
            for jx, (b_, c0_, n_, _pre) in enumerate(jobs):
                last_issued[b_] = c0_ + n_ - 1
                last_jx[b_] = jx
                if b_ not in first_issued:
                    first_issued[b_] = c0_
            e_alls = [
                epool.tile([CH, NCH], F32, tag="eall", name=f"e_all{b}")
                for b in range(BPC)
            ]
            segs = [
                psSeg.tile([K, H + 1], F32, tag="seg", name=f"seg{b}")
                for b in range(BPC)
            ]
            # one shared score-PSUM tile; jobs rotate over 3 column slots
            s_ps_all = psS.tile([CH, 4 * GRP], F32, tag="spsall", name="s_ps_all")
            slot_ctr = [0]

            def dma_pt(b, c0, n):
                pt_tile = ptp.tile([CH, 2, GRP * CH], FP8, tag="pt", name="pt_tile")
                nc.sync.dma_start(
                    out=pt_tile[:, :, 0 : n * CH],
                    in_=bass.AP(
                        projq,
                        b * CH * 2 * T + c0 * CH,
                        [[2 * T, CH], [T, 2], [1, n * CH]],
                    ),
                )
                return pt_tile

            # kick off the first score slab before the constant loads
            pt_tiles = {0: dma_pt(*jobs[0][:3])}

            # ---- constants ----
            tcol = const.tile([CH, NCH], F32)
            nc.gpsimd.iota(
                tcol[:],
                pattern=[[CH, NCH]],
                base=0,
                channel_multiplier=1,
                allow_small_or_imprecise_dtypes=True,
            )
            # wp gates the first W1 matmul: one Act HWDGE slot at the head.
            # The rest go through Pool SWDGE, off the SP/HWDGE input stream.
            wp = const.tile([CH, 2, HQ], FP8)
            nc.scalar.dma_start(
                out=wp[:],
                in_=bass.AP(wpack, 0, [[2 * HQ, CH], [HQ, 2], [1, HQ]]),
            )
            w2_sb = const.tile([HQ, 1], BF16)
            nc.gpsimd.dma_start(out=w2_sb[:], in_=bass.AP(w2t, 0, [[1, HQ], [1, 1]]))
            b1_sb = const.tile([HQ, 1], F32)
            nc.gpsimd.dma_start(out=b1_sb[:], in_=bass.AP(b1, 0, [[1, HQ], [1, 1]]))
            bnd = const.tile([CH, 2, BPC, K], I16)
            nc.gpsimd.dma_start(
                out=bnd[:],
                in_=bass.AP(bounds, 0, [[0, CH], [BPC * K, 2], [K, BPC], [1, K]]),
            )
            if TR_ENABLE:
                wpb_sb = const.tile([CH, 2, HQ], BF16)
                nc.gpsimd.dma_start(
                    out=wpb_sb[:],
                    in_=bass.AP(wpb, 0, [[2 * HQ, CH], [HQ, 2], [1, HQ]]),
                )
                ident = const.tile([CH, CH], BF16)
                nc.gpsimd.dma_start(
                    out=ident[:], in_=bass.AP(identm, 0, [[CH, CH], [1, CH]])
                )

            def scores_part(pt_tile, pt_off, n, sbase):
                # fp8 DoubleRow W1 + tanh + per-chunk W2 into s_ps_all cols
                # [sbase, sbase+n)
                for s0 in range(0, n, SUB):
                    ns = min(SUB, n - s0)
                    po = pt_off + s0
                    hps = psH.tile([HQ, SUB * CH], F32, tag="hps", name="hps")
                    nc.tensor.matmul(
                        hps[:, 0 : ns * CH],
                        wp[:],
                        pt_tile[:, :, po * CH : (po + ns) * CH],
                        start=True,
                        stop=True,
                        perf_mode=mybir.MatmulPerfMode.DoubleRow,
                    )
                    hts = htp.tile([HQ, SUB * CH], BF16, tag="hts", name="hts")
                    nc.scalar.activation(
                        out=hts[:, 0 : ns * CH],
                        in_=hps[:, 0 : ns * CH],
                        func=mybir.ActivationFunctionType.Tanh,
                        bias=b1_sb[:],
                        scale=1.0,
                    )
                    for j in range(ns):
                        nc.tensor.matmul(
                            s_ps_all[:, sbase + s0 + j : sbase + s0 + j + 1],
                            hts[:, j * CH : (j + 1) * CH],
                            w2_sb[:],
                            start=True,
                            stop=True,
                        )

            def scores(b, c0, n, pt_tile, pt_off=0):
                slot = slot_ctr[0] % 4
                slot_ctr[0] += 1
                sbase = slot * GRP
                scores_part(pt_tile, pt_off, n, sbase)
                nc.scalar.activation(
                    out=e_alls[b][:, c0 : c0 + n],
                    in_=s_ps_all[:, sbase : sbase + n],
                    func=mybir.ActivationFunctionType.Exp,
                )

            # --- pipelined transpose-score stages for tr jobs ---
            tr_state = {}

            def tr_stage1(jx):
                # fp8 part (pt fetched 2 beats ago) + PE transposes of the
                # g tile (fetched 1 beat ago) into a bf16 PSUM tile
                b, c0, n, _ = jobs[jx]
                slot = slot_ctr[0] % 4
                slot_ctr[0] += 1
                sbase = slot * GRP
                scores_part(pt_tiles.pop(jx), 0, GRP - TN, sbase + TN)
                g_tile = tr_state[jx]["g"]
                psT_t = psT.tile(
                    [CH, TN * 2 * CH], BF16, tag="pst", name="psT_t"
                )
                for g in range(TN):
                    for h in range(2):
                        nc.tensor.matmul(
                            psT_t[:, (2 * g + h) * CH : (2 * g + h + 1) * CH],
                            g_tile[:, g, h * CH : (h + 1) * CH],
                            ident[:],
                            start=True,
                            stop=True,
                            is_transpose=True,
                        )
                tr_state[jx]["psT"] = psT_t
                tr_state[jx]["sbase"] = sbase

            def tr_stage2(jx):
                # PSUM -> SBUF copy of the transposed halves
                trsb = trp.tile([CH, TN * 2 * CH], BF16, tag="tr", name="trsb")
                eng = COPY_ENGS[jx % len(COPY_ENGS)]
                if eng is nc.scalar:
                    eng.copy(out=trsb[:], in_=tr_state[jx].pop("psT")[:])
                else:
                    eng.tensor_copy(trsb[:], tr_state[jx].pop("psT")[:])
                tr_state[jx]["trsb"] = trsb

            def tr_stage3(jx):
                # bf16 W1 (2 accumulating halves per chunk) + tanh + W2 + exp
                b, c0, n, _ = jobs[jx]
                sbase = tr_state[jx]["sbase"]
                trsb = tr_state[jx].pop("trsb")
                hps = psH.tile([HQ, TN * CH], F32, tag="hps", name="hps_tr")
                for g in range(TN):
                    for h in range(2):
                        nc.tensor.matmul(
                            hps[:, g * CH : (g + 1) * CH],
                            wpb_sb[:, h, :],
                            trsb[:, (2 * g + h) * CH : (2 * g + h + 1) * CH],
                            start=(h == 0),
                            stop=(h == 1),
                        )
                hts = htp.tile([HQ, TN * CH], BF16, tag="hts", name="hts_tr")
                nc.scalar.activation(
                    out=hts[:],
                    in_=hps[:],
                    func=mybir.ActivationFunctionType.Tanh,
                    bias=b1_sb[:],
                    scale=1.0,
                )
                for g in range(TN):
                    nc.tensor.matmul(
                        s_ps_all[:, sbase + g : sbase + g + 1],
                        hts[:, g * CH : (g + 1) * CH],
                        w2_sb[:],
                        start=True,
                        stop=True,
                    )
                nc.scalar.activation(
                    out=e_alls[b][:, c0 : c0 + GRP],
                    in_=s_ps_all[:, sbase : sbase + GRP],
                    func=mybir.ActivationFunctionType.Exp,
                )

            def agen(b, c0, n, hold=False):
                e_all = e_alls[b]
                a2s = []
                for g in range(n):
                    c = c0 + g
                    eng = nc.gpsimd if (c % POOL_MOD < POOL_TAKE) else nc.vector
                    cmp = apool.tile([CH, 2, K], BF16, tag="a1", bufs=10, name="cmp")
                    a2 = apool.tile(
                        [CH, K], BF16, tag="a2h" if hold else "a2",
                        bufs=16 if hold else 32, name="a2",
                    )
                    eng.tensor_scalar(
                        out=cmp[:],
                        in0=bnd[:, :, b, :],
                        scalar1=tcol[:, c : c + 1],
                        scalar2=e_all[:, c : c + 1],
                        op0=mybir.AluOpType.is_le,
                        op1=mybir.AluOpType.mult,
                    )
                    eng.tensor_tensor(
                        out=a2[:],
                        in0=cmp[:, 0, :],
                        in1=cmp[:, 1, :],
                        op=mybir.AluOpType.subtract,
                    )
                    a2s.append(a2)
                return a2s

            def dma_g(b, c0, n, tag="g", bufs=None):
                HP = H + 1
                g_tile = projp.tile(
                    [CH, GRP, HP], BF16, tag=tag, name="g_tile",
                    **({"bufs": bufs} if bufs else {}),
                )
                G, g0 = c0 // GRP, c0 % GRP
                nc.sync.dma_start(
                    out=g_tile[:, 0:n, :],
                    in_=bass.AP(
                        proj,
                        (b * (NCH // GRP) + G) * CH * GRP * HP + g0 * HP,
                        [[GRP * HP, CH], [HP, n], [1, HP]],
                    ),
                )
                return g_tile

            def seg_mms(b, c0, n, a2s, g_tile):
                seg = segs[b]
                for g in range(n):
                    c = c0 + g
                    nc.tensor.matmul(
                        seg[:],
                        a2s[g][:],
                        g_tile[:, g, :],
                        start=(c == first_issued[b]),
                        stop=(c == last_issued[b]),
                    )

            def epilogue(b):
                seg = segs[b]
                ot = outp.tile([K, H + 1], F16, name=f"ot{b}")
                nc.scalar.copy(out=ot[:], in_=seg[:])
                return ot

            def out_dma(b, ot):
                nc.sync.dma_start(
                    out=bass.AP(out, b * K * (H + 1), [[H + 1, K], [1, H + 1]]),
                    in_=ot[:],
                )

            # ---- static schedule ----
            pre_set = [jx for jx, j in enumerate(jobs) if j[3]]
            taper_set = [
                jx for jx, j in enumerate(jobs) if not j[3] and j[2] < GRP
            ]
            lead_set = pre_set + taper_set
            last_b = BPC - 1
            c_tap = min(jobs[jx][1] for jx in taper_set)
            # tr jobs: full stream jobs except the first two and each batch's
            # last full stream job (tail safety)
            tr_set = {
                jx
                for jx, j in enumerate(jobs)
                if j[2] == GRP and not j[3] and jx not in (0, 1, 2)
            } if TR_ENABLE else set()
            if TR_ENABLE and TR_ONLY is not None:
                tr_set &= TR_ONLY
            for b_ in range(BPC):
                tr_set.discard(
                    max(
                        jx
                        for jx, j in enumerate(jobs)
                        if j[0] == b_ and j[2] == GRP and not j[3]
                    )
                )
            a2_map = {}
            g_pre = {}
            # lead: scores+masks for job0, the preloaded job, the taper jobs
            # (one fused pass) and stream jobs 1, 2
            for sx in pre_set:
                pt_tiles[sx] = dma_pt(*jobs[sx][:3])
            pt_tap = dma_pt(last_b, c_tap, GRP)
            scores(*jobs[0][:3], pt_tiles.pop(0))
            for sx in pre_set:
                scores(*jobs[sx][:3], pt_tiles.pop(sx))
                a2_map[sx] = agen(*jobs[sx][:3], hold=True)
                g_pre[sx] = dma_g(*jobs[sx][:3], tag="gpre", bufs=2)
            scores(last_b, c_tap, GRP, pt_tap)
            for sx in taper_set:
                a2_map[sx] = agen(*jobs[sx][:3], hold=True)
            for sx in (1, 2):
                pt_tiles[sx] = dma_pt(*jobs[sx][:3])
                scores(*jobs[sx][:3], pt_tiles.pop(sx))
            # fp8-scored stream jobs outside the lead
            sched_pt, sched_sc, sched_agen = {}, {}, {}
            for jx, j in enumerate(jobs):
                if (
                    jx > 2
                    and jx not in tr_set
                    and jx not in lead_set
                    and not j[3]
                ):
                    sched_pt.setdefault(max(jx - 4, 0), []).append(jx)
                    sched_sc.setdefault(max(jx - 3, 0), []).append(jx)
                    sched_agen.setdefault(max(jx - 2, 0), []).append(jx)
            for aj in (0, 1, 2):
                if aj not in a2_map:
                    a2_map[aj] = agen(*jobs[aj][:3])

            def seg_at(jx):
                # jobs after the first tr job defer their seg matmuls 3 beats
                # (keeps per-batch accumulation order monotone)
                if jx in (0, 1, 2) or jobs[jx][3]:
                    return jx
                return jx + 3

            emit_seg = {}
            for jx in range(len(jobs)):
                emit_seg.setdefault(seg_at(jx), []).append(jx)
            copy_at = {seg_at(last_jx[b_]) + 2: b_ for b_ in range(BPC)}
            dma_at = {seg_at(last_jx[b_]) + 4: b_ for b_ in range(BPC)}

            ots = {}
            g_map = {}
            NITER = len(jobs) + 4
            for i in range(NITER):
                # non-tr stream jobs fetch their g at their own iteration,
                # before any seg emission that might consume it this beat
                if i < len(jobs) and i not in tr_set and not jobs[i][3]:
                    g_map[i] = dma_g(*jobs[i][:3])
                for jx in sorted(emit_seg.pop(i, [])):
                    b, c0, n, pre = jobs[jx]
                    if pre:
                        g_tile = g_pre.pop(jx)
                    elif jx in tr_set:
                        g_tile = tr_state.pop(jx)["g"]
                    else:
                        g_tile = g_map.pop(jx)
                    seg_mms(b, c0, n, a2_map.pop(jx), g_tile)
                for sx in sched_pt.pop(i, []):
                    pt_tiles[sx] = dma_pt(*jobs[sx][:3])
                for sx in sched_sc.pop(i, []):
                    scores(*jobs[sx][:3], pt_tiles.pop(sx))
                for sx in sched_agen.pop(i, []):
                    a2_map[sx] = agen(*jobs[sx][:3])
                j = i + 3
                if j < len(jobs) and j in tr_set:
                    pt_tiles[j] = dma_pt(jobs[j][0], jobs[j][1] + TN, GRP - TN)
                j = i + 2
                if j < len(jobs) and j in tr_set:
                    tr_state[j] = {"g": dma_g(*jobs[j][:3])}
                j = i - 1
                if j >= 0 and j in tr_set:
                    tr_stage3(j)
                j = i + 1
                if j < len(jobs) and j in tr_set:
                    tr_stage1(j)
                if i in tr_set:
                    tr_stage2(i)
                j = i - 2
                if j >= 0 and j in tr_set:
                    a2_map[j] = agen(*jobs[j][:3])
                if i in copy_at:
                    ots[copy_at[i]] = epilogue(copy_at[i])
                if i in dma_at:
                    out_dma(dma_at[i], ots[dma_at[i]])
            for b_ in range(BPC):
                if b_ not in ots:
                    ots[b_] = epilogue(b_)
                    out_dma(b_, ots[b_])

    nc.compile()
    return nc


_prog_cache = None
LAST_RESULTS = None


def _get_program():
    global _prog_cache
    if _prog_cache is None:
        _prog_cache = build_program()
    return _prog_cache


def kernel(**inputs):
    proj = np.asarray(inputs["projected"], dtype=np.float32)
    bnds = np.asarray(inputs["boundaries"])
    slot = np.asarray(inputs["slot_mask"])
    W1 = np.asarray(inputs["W1"], dtype=np.float32)
    b1 = np.ascontiguousarray(np.asarray(inputs["b1"], dtype=np.float32))
    W2 = np.asarray(inputs["W2"], dtype=np.float32).reshape(HQ)

    live = slot > 0
    starts = np.where(live, bnds[..., 0], 0).astype(np.int16)     # [B, K]
    ends = np.where(live, bnds[..., 1], 0).astype(np.int16)

    # h-major fp8 for scores: [B, p, half, T]
    projq = np.ascontiguousarray(
        proj.astype(ml_dtypes.float8_e4m3)
        .transpose(0, 2, 1)
        .reshape(B, 2, CH, T)
        .transpose(0, 2, 1, 3)
    )
    # [B, T, H+1] (ones col baked in) -> [B, G, p, g, h]
    proj_bf = np.empty((B, T, H + 1), dtype=ml_dtypes.bfloat16)
    proj_bf[:, :, :H] = proj
    proj_bf[:, :, H] = 1.0
    proj_bf = np.ascontiguousarray(
        proj_bf.reshape(B, NCH // GRP, GRP, CH, H + 1).transpose(0, 1, 3, 2, 4)
    )

    # W1 packed for DoubleRow (fp8) and the transpose path (bf16)
    w1h = np.ascontiguousarray(W1.reshape(2, CH, HQ).transpose(1, 0, 2))
    wpack = w1h.astype(ml_dtypes.float8_e4m3)
    wpb = w1h.astype(ml_dtypes.bfloat16)
    identm = np.eye(CH, dtype=ml_dtypes.bfloat16)
    w2t = W2.astype(ml_dtypes.bfloat16)

    nc = _get_program()
    in_maps = []
    for i in range(NCORES):
        lo, hi = i * BPC, (i + 1) * BPC
        in_maps.append(
            {
                "proj": proj_bf[lo:hi],
                "projq": projq[lo:hi],
                "bounds": np.ascontiguousarray(
                    np.stack([starts[lo:hi], ends[lo:hi]])
                ),
                "wpack": wpack,
                **({"wpb": wpb, "identm": identm} if TR_ENABLE else {}),
                "w2t": w2t,
                "b1": b1,
            }
        )

    res = run_bass_kernel_spmd(nc, in_maps, core_ids=list(range(NCORES)))
    global LAST_RESULTS
    LAST_RESULTS = res
    raw = np.concatenate(
        [np.asarray(r["out"]) for r in res.results], axis=0
    ).astype(np.float32)                                           # [B, K, H+1]
    num, den = raw[..., :H], raw[..., H:]
    return num / np.where(den > 0, den, 1.0)
